# revision 4
# baseline (speedup 1.0000x reference)
"""KNN soft classifier on 8 Trainium2 NeuronCores.

Sharding: data-parallel over queries. Each core gets B/8 = 256 queries and
the full (transposed, padded) memory bank. No collectives needed; the host
slices q and concatenates the per-core [256, 1000] outputs.

Per-core pipeline (all compute on device):
  1. q norms:    ACT square+accum -> sqrt -> DVE reciprocal          (rq [128,1] per m-tile)
  2. feat norms: ACT square, PE ones-matmul (contract over D),
                 ACT sqrt, DVE reciprocal, GPSIMD partition-bcast    (per 512-col subchunk)
  3. scores:     PE matmul qT.T @ fT (fp32), evacuated by one DVE
                 scalar_tensor_tensor: (psum * rq) * rnorm_bcast
  4. local top-8 per 2048-col chunk: DVE max8 + max_index            (Vall/Iall [128, 392])
  5. global top-32 of 392 candidates: 4x (max8 + max_index + match_replace)
  6. label fetch: DVE iota-compare gather of global index, then
                  GPSIMD indirect DMA gather from labels in DRAM
  7. softmax (exp on ACT) + class scatter via DVE is_equal*weight, row-normalize
"""

import numpy as np

B = 2048
D = 512
N = 100000
C = 1000
K = 32
TAU = 0.2
NCORES = 8
BL = B // NCORES          # 256 queries per core
MT = BL // 128            # 2 m-tiles of 128 queries
LCH = 2048                # candidate chunk (columns) per top-8 pass
NCHUNK = (N + LCH - 1) // LCH          # 49
NPAD = NCHUNK * LCH                    # 100352
NTAIL = N - (NCHUNK - 1) * LCH         # 1696 valid cols in last chunk
NCAND = NCHUNK * 8                     # 392 candidates per query
NSUB = LCH // 512                      # 4 subchunks of 512 per chunk
NEG = -1e30

_cache = {}


def _build():
    from contextlib import ExitStack
    import concourse.tile as tile
    from concourse import bacc, mybir
    from concourse.bass import IndirectOffsetOnAxis

    fp32 = mybir.dt.float32
    u32 = mybir.dt.uint32
    u16 = mybir.dt.uint16

    nc = bacc.Bacc("TRN2", target_bir_lowering=False, debug=False,
                   num_devices=NCORES)

    qT_d = nc.dram_tensor("qT", [D, BL], fp32, kind="ExternalInput")
    qrm_d = nc.dram_tensor("qrm", [BL, D], fp32, kind="ExternalInput")
    fT_d = nc.dram_tensor("fT", [D, NPAD], fp32, kind="ExternalInput")
    lab_d = nc.dram_tensor("lab", [NPAD, 1], fp32, kind="ExternalInput")
    base_d = nc.dram_tensor("basec", [128, NCAND], u32, kind="ExternalInput")
    posi_d = nc.dram_tensor("posi", [128, NCAND], fp32, kind="ExternalInput")
    iotc_d = nc.dram_tensor("iotc", [128, C], fp32, kind="ExternalInput")
    out_d = nc.dram_tensor("out", [BL, C], fp32, kind="ExternalOutput")

    with tile.TileContext(nc) as tc, ExitStack() as ctx:
        cpool = ctx.enter_context(tc.tile_pool(name="consts", bufs=1))
        fpool = ctx.enter_context(tc.tile_pool(name="feats", bufs=2))
        spool = ctx.enter_context(tc.tile_pool(name="scores", bufs=3))
        npool = ctx.enter_context(tc.tile_pool(name="norms", bufs=3))
        qpool = ctx.enter_context(tc.tile_pool(name="qtiles", bufs=1))
        vpool = ctx.enter_context(tc.tile_pool(name="cands", bufs=1))
        wpool = ctx.enter_context(tc.tile_pool(name="work", bufs=2))
        ppool = ctx.enter_context(tc.tile_pool(name="psum", bufs=3, space="PSUM"))
        npsum = ctx.enter_context(tc.tile_pool(name="npsum", bufs=2, space="PSUM"))

        # ---- constants / small setup ----
        ones = cpool.tile([128, 1], fp32)
        nc.vector.memset(ones[:], 1.0)
        base_c = cpool.tile([128, NCAND], u32)
        nc.sync.dma_start(base_c[:], base_d.ap())
        posi_c = cpool.tile([128, NCAND], fp32)
        nc.sync.dma_start(posi_c[:], posi_d.ap())
        iot_c = cpool.tile([128, C], fp32)
        nc.sync.dma_start(iot_c[:], iotc_d.ap())

        # query tiles (stationary weights) + query norms
        qT_sb = []
        rq = []
        for m in range(MT):
            qts = []
            for k in range(4):
                t = qpool.tile([128, 128], fp32, tag=f"qT{m}{k}", name=f"qTsb{m}{k}")
                nc.sync.dma_start(t[:], qT_d.ap()[k * 128:(k + 1) * 128,
                                                  m * 128:(m + 1) * 128])
                qts.append(t)
            qT_sb.append(qts)
            qt = qpool.tile([128, D], fp32, tag=f"qrm{m}")
            nc.sync.dma_start(qt[:], qrm_d.ap()[m * 128:(m + 1) * 128, :])
            qsq = qpool.tile([128, D], fp32, tag=f"qsq{m}")
            qn2 = qpool.tile([128, 1], fp32, tag=f"qn2{m}")
            nc.scalar.activation(qsq[:], qt[:], mybir.ActivationFunctionType.Square,
                                 accum_out=qn2[:])
            qn = qpool.tile([128, 1], fp32, tag=f"qn{m}")
            nc.scalar.sqrt(qn[:], qn2[:])
            r = qpool.tile([128, 1], fp32, tag=f"rq{m}")
            nc.vector.reciprocal(r[:], qn[:])
            rq.append(r)

        Vall = [vpool.tile([128, NCAND], fp32, tag=f"Vall{m}", name=f"Vall{m}") for m in range(MT)]
        Iall = [vpool.tile([128, NCAND], u32, tag=f"Iall{m}", name=f"Iall{m}") for m in range(MT)]

        # ---- main streaming loop over candidate chunks ----
        for cix in range(NCHUNK):
            ft = [fpool.tile([128, LCH], fp32, tag=f"ft{k}", name=f"ft{k}_{cix}") for k in range(4)]
            for k in range(4):
                nc.sync.dma_start(ft[k][:], fT_d.ap()[k * 128:(k + 1) * 128,
                                                      cix * LCH:(cix + 1) * LCH])
            S = [spool.tile([128, LCH], fp32, tag=f"S{m}", name=f"S{m}_{cix}") for m in range(MT)]
            for s in range(NSUB):
                sl = slice(s * 512, (s + 1) * 512)
                # feat norms for this subchunk
                n2p = npsum.tile([1, 512], fp32, tag="n2p")
                for k in range(4):
                    sq = npool.tile([128, 512], fp32, tag="sq")
                    nc.scalar.square(sq[:], ft[k][:, sl])
                    nc.tensor.matmul(n2p[:], ones[:], sq[:],
                                     start=(k == 0), stop=(k == 3))
                rt = npool.tile([1, 512], fp32, tag="rt")
                nc.scalar.sqrt(rt[:], n2p[:])
                rn = npool.tile([1, 512], fp32, tag="rn")
                nc.vector.reciprocal(rn[:], rt[:])
                rnb = npool.tile([128, 512], fp32, tag="rnb")
                nc.gpsimd.partition_broadcast(rnb[:], rn[:])
                for m in range(MT):
                    ps = ppool.tile([128, 512], fp32, tag=f"ps{m}")
                    for k in range(4):
                        nc.tensor.matmul(ps[:], qT_sb[m][k][:], ft[k][:, sl],
                                         start=(k == 0), stop=(k == 3))
                    nc.vector.scalar_tensor_tensor(
                        S[m][:, sl], ps[:], rq[m][:], rnb[:],
                        op0=mybir.AluOpType.mult, op1=mybir.AluOpType.mult)
            if cix == NCHUNK - 1 and NTAIL < LCH:
                for m in range(MT):
                    nc.vector.memset(S[m][:, NTAIL:], NEG)
            for m in range(MT):
                v8 = wpool.tile([128, 8], fp32, tag="v8")
                nc.vector.max(out=v8[:], in_=S[m][:])
                nc.vector.tensor_copy(Vall[m][:, cix * 8:(cix + 1) * 8], v8[:])
                nc.vector.max_index(out=Iall[m][:, cix * 8:(cix + 1) * 8],
                                    in_max=v8[:], in_values=S[m][:])

        # ---- per m-tile: global top-32, labels, softmax scatter ----
        for m in range(MT):
            gidx_u = wpool.tile([128, NCAND], u32, tag="gidxu")
            nc.vector.tensor_add(gidx_u[:], Iall[m][:], base_c[:])
            gidx_f = wpool.tile([128, NCAND], fp32, tag="gidxf")
            nc.vector.tensor_copy(gidx_f[:], gidx_u[:])

            vtop = wpool.tile([128, K], fp32, tag="vtop")
            pos = wpool.tile([128, K], u16, tag="pos")
            for r in range(4):
                sl8 = slice(r * 8, (r + 1) * 8)
                nc.vector.max(out=vtop[:, sl8], in_=Vall[m][:])
                nc.vector.max_index(out=pos[:, sl8], in_max=vtop[:, sl8],
                                    in_values=Vall[m][:])
                if r < 3:
                    nc.vector.match_replace(out=Vall[m][:],
                                            in_to_replace=vtop[:, sl8],
                                            in_values=Vall[m][:], imm_value=NEG)
            pos_f = wpool.tile([128, K], fp32, tag="posf")
            nc.vector.tensor_copy(pos_f[:], pos[:])
            gsel_f = wpool.tile([128, K], fp32, tag="gself")
            scr = wpool.tile([128, NCAND], fp32, tag="scr")
            for k in range(K):
                nc.vector.scalar_tensor_tensor(
                    scr[:], posi_c[:], pos_f[:, k:k + 1], gidx_f[:],
                    op0=mybir.AluOpType.is_equal, op1=mybir.AluOpType.mult,
                    accum_out=gsel_f[:, k:k + 1])
            gsel_u = wpool.tile([128, K], u32, tag="gselu")
            nc.vector.tensor_copy(gsel_u[:], gsel_f[:])
            lab_sb = wpool.tile([128, K], fp32, tag="labsb")
            # HW indirect DMA consumes ONE offset per partition row (gathers a
            # row per partition), so issue one call per selected column.
            for k in range(K):
                nc.gpsimd.indirect_dma_start(
                    out=lab_sb[:, k:k + 1], out_offset=None, in_=lab_d.ap(),
                    in_offset=IndirectOffsetOnAxis(ap=gsel_u[:, k:k + 1], axis=0))

            # softmax weights: u = exp((v - max)/tau); max = vtop[:,0]
            nbias = wpool.tile([128, 1], fp32, tag="nbias")
            nc.scalar.mul(nbias[:], vtop[:, 0:1], -1.0 / TAU)
            uw = wpool.tile([128, K], fp32, tag="uw")
            nc.scalar.activation(uw[:], vtop[:], mybir.ActivationFunctionType.Exp,
                                 bias=nbias[:], scale=1.0 / TAU)
            acc = wpool.tile([128, C], fp32, tag="acc")
            tmp = wpool.tile([128, C], fp32, tag="tmpc")
            for k in range(K):
                dst = acc if k == 0 else tmp
                nc.vector.tensor_scalar(dst[:], iot_c[:], lab_sb[:, k:k + 1],
                                        uw[:, k:k + 1],
                                        op0=mybir.AluOpType.is_equal,
                                        op1=mybir.AluOpType.mult)
                if k > 0:
                    nc.vector.tensor_add(acc[:], acc[:], tmp[:])
            ssum = wpool.tile([128, 1], fp32, tag="ssum")
            nc.vector.reduce_sum(ssum[:], acc[:], axis=mybir.AxisListType.X)
            rs = wpool.tile([128, 1], fp32, tag="rs")
            nc.vector.reciprocal(rs[:], ssum[:])
            outt = wpool.tile([128, C], fp32, tag="outt")
            nc.vector.tensor_scalar(outt[:], acc[:], rs[:], None,
                                    op0=mybir.AluOpType.mult)
            nc.sync.dma_start(out_d.ap()[m * 128:(m + 1) * 128, :], outt[:])

    nc.compile()
    return nc


def _prep_inputs(q, feats, labels):
    qn = np.ascontiguousarray(q, dtype=np.float32)
    f = np.ascontiguousarray(feats, dtype=np.float32)
    fpad = np.empty((NPAD, D), dtype=np.float32)
    fpad[:N] = f
    fpad[N:] = f[0]                      # harmless pad (masked out on device)
    fT = np.ascontiguousarray(fpad.T)    # [D, NPAD]
    lab = np.zeros((NPAD, 1), dtype=np.float32)
    lab[:N, 0] = labels.astype(np.float32)
    base = np.broadcast_to(
        (np.arange(NCAND, dtype=np.uint32) // 8 * LCH)[None, :], (128, NCAND))
    base = np.ascontiguousarray(base)
    posi = np.ascontiguousarray(np.broadcast_to(
        np.arange(NCAND, dtype=np.float32)[None, :], (128, NCAND)))
    iotc = np.ascontiguousarray(np.broadcast_to(
        np.arange(C, dtype=np.float32)[None, :], (128, C)))
    in_maps = []
    for c in range(NCORES):
        qs = qn[c * BL:(c + 1) * BL]
        in_maps.append({
            "qT": np.ascontiguousarray(qs.T),
            "qrm": qs,
            "fT": fT,
            "lab": lab,
            "basec": base,
            "posi": posi,
            "iotc": iotc,
        })
    return in_maps


def kernel(q, feats, labels):
    from concourse.bass_utils import run_bass_kernel_spmd
    if "nc" not in _cache:
        _cache["nc"] = _build()
    nc = _cache["nc"]
    in_maps = _prep_inputs(q, feats, labels)
    res = run_bass_kernel_spmd(nc, in_maps, list(range(NCORES)))
    out = np.concatenate([res.results[c]["out"] for c in range(NCORES)], axis=0)
    return out.astype(np.float32)
